# revision 10
# baseline (speedup 1.0000x reference)
"""Causal single-head attention (B=4, T=4096, C=1024, H=64) on 8 trn2 NeuronCores.

Sharding: core = (batch b = core//2, parity p = core%2). Each core owns the
interleaved context tiles {p, p+2, ...} of its batch (balanced under the causal
mask) and computes partial flash-attention (numerator + denominator) for ALL
queries of the batch; the host sums the two partials per batch and divides.

v2 over the baseline:
  - x is shipped slice-major ([128, sl*4096 + j*512 + t]) so each slice DMA is
    one 8KB-contiguous segment per partition.
  - partner-half x arrives as fp8e4; partner K^T is projected with DoubleRow
    fp8 matmuls (half the PE slots), bias added in fp32.
  - PV uses a single PSUM chain per query block: off-diagonal (and diagonal
    for tb>=1) probabilities are written as fp8e4 by the ACT exp and consumed
    by DoubleRow matmuls against an fp8 V_aug copy — one instruction per
    context-tile pair instead of two full bf16 matmuls. Only block 0 (queries
    with <512 context) keeps the bf16 V path for accuracy.
  - scores stay bf16 (row-packed 64-contraction pairs, as before).
"""

import sys

for _p in ("/root/.axon_site/_ro/trn_rl_repo", "/root/.axon_site/_ro/pypackages"):
    if _p not in sys.path:
        sys.path.append(_p)

import ml_dtypes
import numpy as np

import concourse.bass as bass
import concourse.mybir as mybir
import concourse.tile as tile
from concourse import bacc
from concourse.bass_utils import run_bass_kernel_spmd
from concourse.masks import make_identity

B, T, C, H = 4, 4096, 1024, 64
N_CORES = 8
SCALE = C ** -0.5
F32 = mybir.dt.float32
BF16 = mybir.dt.bfloat16
FP8 = mybir.dt.float8e4
NP_FP8 = ml_dtypes.float8_e4m3


def build_kernel(t_full=T):
    """Build the SPMD Bass/Tile program for sequence length t_full."""
    t_own = t_full // 2           # context rows owned by this core
    n_own = t_own // 128          # own 128-row s-tiles
    n_cchunk = C // 128           # contraction chunks of 128
    n_vq = t_own // 512           # own (and partner) 512-col slices

    nc = bacc.Bacc("TRN2", target_bir_lowering=False, debug=False,
                   num_devices=N_CORES)

    xt_d = nc.dram_tensor("xt_own", [128, n_cchunk * 512], BF16,
                          kind="ExternalInput").ap()
    x8o_d = (nc.dram_tensor("x8_own", [128, (n_vq - 1) * n_cchunk * 512], FP8,
                            kind="ExternalInput").ap() if n_vq > 1 else None)
    x8_d = nc.dram_tensor("x8_part", [128, n_vq * n_cchunk * 512], FP8,
                          kind="ExternalInput").ap()
    aux_w = n_cchunk * 128 + n_cchunk * H + 1024
    aux_d = nc.dram_tensor("aux", [128, aux_w], BF16,
                           kind="ExternalInput").ap()
    aux8_w = (n_cchunk // 2) * 128 + 1024
    aux8_d = nc.dram_tensor("aux8", [128, aux8_w], FP8,
                            kind="ExternalInput").ap()
    bkq_d = nc.dram_tensor("bkq", [128], F32, kind="ExternalInput").ap()
    bv_d = nc.dram_tensor("bv", [64], F32, kind="ExternalInput").ap()
    out_d = nc.dram_tensor("out_part", [H + 1, t_full], F32,
                           kind="ExternalOutput").ap()

    with tile.TileContext(nc) as tc:
        with (
            tc.tile_pool(name="persist", bufs=1) as pp,
            tc.tile_pool(name="psB", bufs=2, space="PSUM") as psb,
            tc.tile_pool(name="psS", bufs=2, space="PSUM") as pss,
            tc.tile_pool(name="psO", bufs=2, space="PSUM") as pso,
            tc.tile_pool(name="pt8p", bufs=4) as pt8p,
            tc.tile_pool(name="ptdp", bufs=2) as ptdp,
            tc.tile_pool(name="outp", bufs=2) as outp,
        ):
            # ---- persistent SBUF tensors ----
            xt = pp.tile([128, n_vq * n_cchunk * 512], BF16)   # own x^T
            x8o = (pp.tile([128, (n_vq - 1) * n_cchunk * 512], FP8,
                           name="x8o_sb") if n_vq > 1 else None)  # staging
            x8 = pp.tile([128, n_vq * n_cchunk * 512], FP8)    # partner x^T
            kqT = pp.tile([128, t_full], BF16)   # 0:64 K^T full; 64:128 Q^T own
            sd_hi = pp.tile([128, t_full], BF16)  # K^T dup at partitions 64:128
            gt_lo = pp.tile([64, t_own], BF16)    # Q^T own dup at partitions 0:64
            vT = pp.tile([64, t_own], F32)        # V^T own
            v_sb = pp.tile([128, 256], BF16)      # V_aug tiles 0,1 (bf16, padded)
            v8 = pp.tile([128, n_own * 128], FP8)  # V_aug all tiles (fp8, padded)
            aux_sb = pp.tile([128, aux_w], BF16)
            wkq_sb = aux_sb[:, 0:n_cchunk * 128]
            wv_sb = aux_sb[:, n_cchunk * 128:n_cchunk * 128 + n_cchunk * H]
            mask0 = aux_sb[:, aux_w - 1024:aux_w - 512]
            mask1 = aux_sb[:, aux_w - 512:aux_w]
            aux8_sb = pp.tile([128, aux8_w], FP8)
            wk8_sb = aux8_sb[:, 0:(n_cchunk // 2) * 128]
            mask0_8 = aux8_sb[:, aux8_w - 1024:aux8_w - 512]
            mask1_8 = aux8_sb[:, aux8_w - 512:aux8_w]
            bias_kq = pp.tile([128, 1], F32)
            bias_v = pp.tile([64, 1], F32)
            ident = pp.tile([128, 128], F32)

            make_identity(nc, ident[:, :])

            xt_v = xt[:, :].rearrange("p (s j t) -> p s j t", j=n_cchunk, t=512)
            x8_v = x8[:, :].rearrange(
                "p (s g e t) -> p s g e t", g=n_cchunk // 2, e=2, t=512)
            wk8_v = wk8_sb.rearrange("p (g e h) -> p g e h", e=2, h=H)
            v8_v = v8[:, :].rearrange("p (i c) -> p i c", c=128)

            # wkq gates the first projection: put it on the sync queue AHEAD
            # of the x stream (the scalar HWDGE queue gets starved by the
            # big x transfers); everything else small goes on scalar
            nc.sync.dma_start(out=aux_sb[:, 0:n_cchunk * 128],
                              in_=aux_d[:, 0:n_cchunk * 128])
            nc.scalar.dma_start(out=bias_kq[:, 0:1], in_=bkq_d[:, None])
            nc.scalar.dma_start(out=bias_v[:, 0:1], in_=bv_d[:, None])
            nc.scalar.dma_start(out=aux8_sb[:, :], in_=aux8_d)
            nc.scalar.dma_start(out=aux_sb[:, n_cchunk * 128:aux_w],
                                in_=aux_d[:, n_cchunk * 128:aux_w])

            # V_aug padding: zero cols, ones at col 64 of each slot
            nc.gpsimd.memset(v8[:, :], 0.0)
            nc.gpsimd.memset(v_sb[:, :], 0.0)
            nc.vector.tensor_scalar(
                v8_v[:, :, 64], ident[:, 0:n_own], 0.0, 1.0,
                op0=mybir.AluOpType.mult, op1=mybir.AluOpType.add)
            nc.vector.tensor_scalar(
                v_sb[:, :].rearrange("p (i c) -> p i c", c=128)[:, :, 64],
                ident[:, 0:2], 0.0, 1.0,
                op0=mybir.AluOpType.mult, op1=mybir.AluOpType.add)

            def load_own(sl):
                if sl == 0:
                    # split so the first chunks land (and the first
                    # projection starts) as early as possible
                    nc.sync.dma_start(out=xt[:, 0:1024], in_=xt_d[:, 0:1024])
                    nc.sync.dma_start(out=xt[:, 1024:2048],
                                      in_=xt_d[:, 1024:2048])
                    nc.sync.dma_start(out=xt[:, 2048:4096],
                                      in_=xt_d[:, 2048:4096])
                    return
                # own slices 1..3 arrive as fp8 (half the DMA bytes of the
                # stream that gates stages 1-2) and are upcast to bf16 on
                # gpsimd before the bf16 projections consume them
                c0, c1 = 4096 * (sl - 1), 4096 * sl
                nc.sync.dma_start(out=x8o[:, c0:c1], in_=x8o_d[:, c0:c1])

            def upcast_own(sl):
                c0, c1 = 4096 * (sl - 1), 4096 * sl
                nc.gpsimd.tensor_copy(
                    xt[:, 4096 * sl:4096 * (sl + 1)], x8o[:, c0:c1])

            def load_partner(sl):
                nc.sync.dma_start(
                    out=x8[:, 4096 * sl:4096 * (sl + 1)],
                    in_=x8_d[:, 4096 * sl:4096 * (sl + 1)])

            def project_kq_own(sl):
                # fused [wk|wq] bf16 projection of own slice sl
                ps = psb.tile([128, 512], F32, tag="psB")
                for j in range(n_cchunk):
                    nc.tensor.matmul(
                        ps[:, :], wkq_sb[:, 128 * j:128 * (j + 1)],
                        xt_v[:, sl, j], start=(j == 0),
                        stop=(j == n_cchunk - 1))
                nc.vector.tensor_scalar_add(
                    kqT[:, 512 * sl:512 * (sl + 1)], ps[:, :], bias_kq[:, 0:1])
                nc.gpsimd.dma_start(
                    sd_hi[64:128, 512 * sl:512 * (sl + 1)],
                    kqT[0:64, 512 * sl:512 * (sl + 1)])
                nc.gpsimd.dma_start(
                    gt_lo[:, 512 * sl:512 * (sl + 1)],
                    kqT[64:128, 512 * sl:512 * (sl + 1)])

            def project_k_partner(sl):
                # fp8 DoubleRow projection: K only, chunk pairs
                ps = psb.tile([64, 512], F32, tag="psB")
                for g in range(n_cchunk // 2):
                    nc.tensor.matmul(
                        ps[:, :], wk8_v[:, g], x8_v[:, sl, g],
                        start=(g == 0), stop=(g == n_cchunk // 2 - 1),
                        perf_mode=mybir.MatmulPerfMode.DoubleRow)
                c0 = t_own + 512 * sl
                nc.vector.tensor_scalar_add(
                    kqT[0:64, c0:c0 + 512], ps[:, :], bias_kq[0:64, 0:1])
                nc.gpsimd.dma_start(
                    sd_hi[64:128, c0:c0 + 512], kqT[0:64, c0:c0 + 512])

            def project_v(sl):
                ps = psb.tile([64, 512], F32, tag="psB")
                for j in range(n_cchunk):
                    nc.tensor.matmul(
                        ps[:, :], wv_sb[:, H * j:H * (j + 1)],
                        xt_v[:, sl, j], start=(j == 0),
                        stop=(j == n_cchunk - 1))
                nc.vector.tensor_scalar_add(
                    vT[:, 512 * sl:512 * (sl + 1)], ps[:, :], bias_v[:, 0:1])

            def v_transpose(i):
                ps = psb.tile([128, 64], F32, tag="psB")
                nc.tensor.transpose(
                    ps[:, :], vT[:, 128 * i:128 * (i + 1)], ident[0:64, 0:64])
                nc.vector.tensor_copy(v8[:, 128 * i:128 * i + 64], ps[:, :])
                if i < 2:
                    nc.vector.tensor_copy(v_sb[:, 128 * i:128 * i + 64],
                                          ps[:, :])

            kq_lo = kqT[0:64, :].rearrange("p (h t) -> p h t", h=2)
            sd_v = sd_hi[64:128, :].rearrange("p (h t) -> p h t", h=2)

            # ---- attention units, software-pipelined with lag-1 PV ----
            # Emitting scores(unit j+1) BEFORE pv(unit j) gives the ACT exp
            # of unit j a full score-pair of tensor time to finish, so the
            # PV matmul never stalls and the PE keeps its pstate/duty.
            po_of = {}

            def emit_scores(tb, ip):
                i0, i1 = 2 * ip, 2 * ip + 1
                ps = pss.tile([128, 1024], F32, tag="psS")
                nc.tensor.matmul(
                    ps[:, 0:512],
                    gt_lo[:, 128 * i0:128 * (i0 + 1)],
                    kq_lo[:, :, 256 * tb:256 * (tb + 1)],
                    start=True, stop=True, tile_position=(0, 0))
                nc.tensor.matmul(
                    ps[:, 512:1024],
                    kqT[64:128, 128 * i1:128 * (i1 + 1)],
                    sd_v[:, :, 256 * tb:256 * (tb + 1)],
                    start=True, stop=True, tile_position=(64, 0))
                return ps

            def emit_exp(tb, ip, ps):
                if tb == 0:
                    pt = ptdp.tile([128, 1024], BF16, tag="ptd")
                else:
                    pt = pt8p.tile([128, 1024], FP8, tag="pt8")
                nc.scalar.activation(
                    pt[:, :], ps[:, :],
                    mybir.ActivationFunctionType.Exp, scale=SCALE)
                if ip == tb:
                    m0 = mask0 if tb == 0 else mask0_8
                    m1 = mask1 if tb == 0 else mask1_8
                    nc.vector.tensor_mul(pt[:, 0:512], pt[:, 0:512], m0)
                    nc.vector.tensor_mul(pt[:, 512:1024], pt[:, 512:1024], m1)
                return pt

            def emit_pv(tb, ip, pt):
                if ip == 0:
                    po_of[tb] = pso.tile([128, 512], F32, tag="psO",
                                         name=f"po{tb}")
                po = po_of[tb]
                if tb == 0:
                    # stationary trimmed to 65 cols: the 63 pad columns
                    # would burn PE power the throttler charges us for
                    nc.tensor.matmul(
                        po[0:65, :], v_sb[:, 0:65], pt[:, 0:512],
                        start=True, stop=False)
                    nc.tensor.matmul(
                        po[0:65, :], v_sb[:, 128:193], pt[:, 512:1024],
                        start=False, stop=True)
                else:
                    i0 = 2 * ip
                    nc.tensor.matmul(
                        po[0:65, :],
                        v8[:, 128 * i0:128 * (i0 + 2)].rearrange(
                            "p (e c) -> p e c", e=2)[:, :, 0:65],
                        pt[:, :].rearrange("p (e t) -> p e t", e=2),
                        start=(ip == 0), stop=(ip == tb),
                        perf_mode=mybir.MatmulPerfMode.DoubleRow)
                if ip == tb:
                    ob = outp.tile([65, 512], F32, tag="ob")
                    nc.vector.tensor_copy(ob[:, :], po[0:65, :])
                    nc.gpsimd.dma_start(
                        out=out_d[:, 512 * tb:512 * (tb + 1)], in_=ob[:, :])
                    del po_of[tb]

            pending = None

            def stream_unit(tb, ip):
                nonlocal pending
                ps = emit_scores(tb, ip)
                pt = emit_exp(tb, ip, ps)
                if pending is not None:
                    emit_pv(*pending)
                pending = (tb, ip, pt)

            # ---- staged pipeline ----
            for k in range(n_vq):
                load_own(k)
                load_partner(k)
            for k in range(n_vq):
                project_kq_own(k)
                project_k_partner(k)
                if k + 1 < n_vq:
                    upcast_own(k + 1)
                project_v(k)
                for i in range(4 * k, min(4 * (k + 1), n_own)):
                    v_transpose(i)
                blocks = ((2 * k + 1, 2 * k) if k == n_vq - 1
                          else (2 * k, 2 * k + 1))
                for tb in blocks:
                    for ip in range(tb + 1):
                        stream_unit(tb, ip)
            emit_pv(*pending)

    nc.compile()
    return nc


def make_core_inputs(x, Wk, bk, Wq, bq, Wv, bv, t_full=T):
    """Shard FULL inputs into the 8 per-core input dicts (layout prep only)."""
    n_tiles = t_full // 128
    n_cchunk = C // 128
    t_own = t_full // 2
    n_vq = t_own // 512
    Wk = np.asarray(Wk, np.float32)
    Wq = np.asarray(Wq, np.float32)
    Wv = np.asarray(Wv, np.float32)
    wkq = np.empty((128, n_cchunk * 128), np.float32)
    wvf = np.empty((128, n_cchunk * H), np.float32)
    for j in range(n_cchunk):
        wkq[:, 128 * j:128 * j + 64] = Wk[128 * j:128 * (j + 1), :]
        wkq[:, 128 * j + 64:128 * (j + 1)] = Wq[128 * j:128 * (j + 1), :]
        wvf[:, H * j:H * (j + 1)] = Wv[128 * j:128 * (j + 1), :]
    # wk8: chunk pairs (2g, 2g+1) side by side, fp8
    wk8 = np.empty((128, (n_cchunk // 2) * 128), np.float32)
    for g in range(n_cchunk // 2):
        wk8[:, 128 * g:128 * g + 64] = Wk[128 * (2 * g):128 * (2 * g + 1), :]
        wk8[:, 128 * g + 64:128 * (g + 1)] = \
            Wk[128 * (2 * g + 1):128 * (2 * g + 2), :]
    bkq = np.concatenate([np.asarray(bk, np.float32),
                          np.asarray(bq, np.float32)])

    def slice_major(mat):
        # mat [t_own, C] -> [128, n_vq*n_cchunk*512] with
        # out[p, 4096*sl + 512*j + t] = mat[512*sl + t, 128*j + p]
        a = mat.reshape(n_vq, 512, n_cchunk, 128)
        return np.ascontiguousarray(
            a.transpose(3, 0, 2, 1).reshape(128, n_vq * n_cchunk * 512))

    ins = []
    for core in range(N_CORES):
        b, p = core // 2, core % 2
        own = np.concatenate(
            [x[b, 128 * j:128 * (j + 1), :] for j in range(p, n_tiles, 2)],
            axis=0)
        part = np.concatenate(
            [x[b, 128 * j:128 * (j + 1), :] for j in range(1 - p, n_tiles, 2)],
            axis=0)
        # mask[m][r, c]: own s-tile (local parity m, abs tile 4tb+2m+p) vs
        # query sub-tile c//128 (abs tile 4tb + A[c//128]); valid iff s <= t
        A = [p, 2 + p, 1 - p, 3 - p]
        masks = np.zeros((2, 128, 512), np.float32)
        rr = np.arange(128)[:, None]
        for m in (0, 1):
            for sub in range(4):
                cz = np.arange(128)[None, :]
                s_abs = 128 * (2 * m + p) + rr
                t_abs = 128 * A[sub] + cz
                masks[m, :, 128 * sub:128 * (sub + 1)] = (s_abs <= t_abs)
        aux = np.concatenate([wkq, wvf, masks[0], masks[1]], axis=1)
        aux8 = np.concatenate([wk8, masks[0], masks[1]], axis=1)
        own_sm = slice_major(own)
        d = {
            "xt_own": own_sm[:, 0:n_cchunk * 512].astype(ml_dtypes.bfloat16),
            "x8_part": slice_major(part).astype(NP_FP8),
            "aux": aux.astype(ml_dtypes.bfloat16),
            "aux8": aux8.astype(NP_FP8),
            "bkq": bkq, "bv": np.asarray(bv, np.float32),
        }
        if t_full // 2 // 512 > 1:
            d["x8_own"] = own_sm[:, n_cchunk * 512:].astype(NP_FP8)
        ins.append(d)
    return ins


def _col_perm(p, t_full):
    """stored column -> absolute t index for a core with parity p."""
    A = [p, 2 + p, 1 - p, 3 - p]
    perm = np.empty(t_full, np.int64)
    for tb in range(t_full // 512):
        for sub in range(4):
            a = 128 * (4 * tb + A[sub])
            s = 512 * tb + 128 * sub
            perm[s:s + 128] = np.arange(a, a + 128)
    return perm


def combine_outputs(parts, t_full=T):
    """parts: list of 8 arrays [H+1, t_full] -> full output [B, t_full, H]."""
    out = np.empty((B, t_full, H), np.float32)
    for b in range(B):
        acc = np.zeros((H + 1, t_full), np.float32)
        for p in (0, 1):
            perm = _col_perm(p, t_full)
            acc[:, perm] += parts[2 * b + p]
        out[b] = (acc[:H, :] / acc[H:H + 1, :]).T
    return out


_NC_CACHE = {}


def kernel(x, Wk, bk, Wq, bq, Wv, bv):
    x = np.asarray(x, np.float32)
    t_full = x.shape[1]
    if t_full not in _NC_CACHE:
        _NC_CACHE[t_full] = build_kernel(t_full)
    nc = _NC_CACHE[t_full]
    ins = make_core_inputs(x, Wk, bk, Wq, bq, Wv, bv, t_full)
    res = run_bass_kernel_spmd(nc, ins, list(range(N_CORES)))
    parts = [res.results[i]["out_part"] for i in range(N_CORES)]
    return combine_outputs(parts, t_full)


if __name__ == "__main__":
    rng = np.random.default_rng(0)
    x = rng.standard_normal((B, T, C), dtype=np.float32)
    Wk = rng.standard_normal((C, H), dtype=np.float32) * SCALE
    Wq = rng.standard_normal((C, H), dtype=np.float32) * SCALE
    Wv = rng.standard_normal((C, H), dtype=np.float32) * SCALE
    bk = rng.standard_normal(H).astype(np.float32) * 0.02
    bq = rng.standard_normal(H).astype(np.float32) * 0.02
    bv = rng.standard_normal(H).astype(np.float32) * 0.02
    out = kernel(x=x, Wk=Wk, bk=bk, Wq=Wq, bq=bq, Wv=Wv, bv=bv)
    print(out.shape, out.dtype)


# revision 16
# speedup vs baseline: 1.4931x; 1.4931x over previous
"""Causal single-head attention (B=4, T=4096, C=1024, H=64) on 8 trn2 NeuronCores.

Sharding: core = (batch b = core//2, parity p = core%2). Each core owns the
interleaved context tiles {p, p+2, ...} of its batch (balanced under the causal
mask) and computes partial flash-attention (numerator + denominator) for ALL
queries of the batch; the host sums the two partials per batch and divides.

Over the original baseline:
  - x is shipped slice-major ([128, sl*4096 + j*512 + t]) so each slice DMA is
    one 8KB-contiguous segment per partition; slice 0 is split 3 ways so the
    first projection starts as soon as ~256KB have landed.
  - partner-half x arrives as fp8e4; partner K^T is projected with DoubleRow
    fp8 matmuls (half the PE slots), bias added in fp32.
  - PV uses a single PSUM chain per query block: off-diagonal (and diagonal
    for tb>=1) probabilities are written as fp8e4 by the ACT exp and consumed
    by DoubleRow matmuls against an fp8 V_aug copy — one instruction per
    context-tile pair instead of two full bf16 matmuls. Only block 0 (queries
    with <512 context) keeps the bf16 V path for accuracy. The V_aug
    stationary is trimmed to 65 columns (V|ones) — the 128-col padding burned
    PE power that the hardware utilization throttler charges for.
  - scores stay bf16 (row-packed 64-contraction pairs): fp8 everywhere was
    tried and REGRESSES — dense fp8 DoubleRow work trips the PE power
    throttle (util limit drops 70%->53%) and all matmuls slow down.
  - the attention stream is software-pipelined with lag-1 PV: scores of unit
    j+1 are emitted before the PV of unit j, so PV never stalls on the
    just-issued exp and the PE keeps its DVFS pstate.
  - diagonal-block exp skips the two query subtiles that are causally invalid
    for both parities (memset instead), trimming the ACT pacer.
"""

import sys

for _p in ("/root/.axon_site/_ro/trn_rl_repo", "/root/.axon_site/_ro/pypackages"):
    if _p not in sys.path:
        sys.path.append(_p)

import ml_dtypes
import numpy as np

import concourse.bass as bass
import concourse.mybir as mybir
import concourse.tile as tile
from concourse import bacc
from concourse.bass_utils import run_bass_kernel_spmd
from concourse.masks import make_identity

B, T, C, H = 4, 4096, 1024, 64
N_CORES = 8
SCALE = C ** -0.5
F32 = mybir.dt.float32
BF16 = mybir.dt.bfloat16
FP8 = mybir.dt.float8e4
NP_FP8 = ml_dtypes.float8_e4m3


def build_kernel(t_full=T):
    """Build the SPMD Bass/Tile program for sequence length t_full."""
    t_own = t_full // 2           # context rows owned by this core
    n_own = t_own // 128          # own 128-row s-tiles
    n_cchunk = C // 128           # contraction chunks of 128
    n_vq = t_own // 512           # own (and partner) 512-col slices

    nc = bacc.Bacc("TRN2", target_bir_lowering=False, debug=False,
                   num_devices=N_CORES)

    xt_d = nc.dram_tensor("xt_own", [128, n_vq * n_cchunk * 512], BF16,
                          kind="ExternalInput").ap()
    x8_d = nc.dram_tensor("x8_part", [128, n_vq * n_cchunk * 512], FP8,
                          kind="ExternalInput").ap()
    aux_w = n_cchunk * 128 + n_cchunk * H + 1024
    aux_d = nc.dram_tensor("aux", [128, aux_w], BF16,
                           kind="ExternalInput").ap()
    aux8_w = (n_cchunk // 2) * 128 + 1024
    aux8_d = nc.dram_tensor("aux8", [128, aux8_w], FP8,
                            kind="ExternalInput").ap()
    bkq_d = nc.dram_tensor("bkq", [128], F32, kind="ExternalInput").ap()
    bv_d = nc.dram_tensor("bv", [64], F32, kind="ExternalInput").ap()
    out_d = nc.dram_tensor("out_part", [H + 1, t_full], F32,
                           kind="ExternalOutput").ap()

    with tile.TileContext(nc) as tc:
        with (
            tc.tile_pool(name="persist", bufs=1) as pp,
            tc.tile_pool(name="psB", bufs=2, space="PSUM") as psb,
            tc.tile_pool(name="psS", bufs=2, space="PSUM") as pss,
            tc.tile_pool(name="psO", bufs=2, space="PSUM") as pso,
            tc.tile_pool(name="pt8p", bufs=4) as pt8p,
            tc.tile_pool(name="ptdp", bufs=2) as ptdp,
            tc.tile_pool(name="outp", bufs=2) as outp,
        ):
            # ---- persistent SBUF tensors ----
            xt = pp.tile([128, n_vq * n_cchunk * 512], BF16)   # own x^T
            x8 = pp.tile([128, n_vq * n_cchunk * 512], FP8)    # partner x^T
            kqT = pp.tile([128, t_full], BF16)   # 0:64 K^T full; 64:128 Q^T own
            sd_hi = pp.tile([128, t_full], BF16)  # K^T dup at partitions 64:128
            gt_lo = pp.tile([64, t_own], BF16)    # Q^T own dup at partitions 0:64
            vT = pp.tile([64, t_own], F32)        # V^T own
            v_sb = pp.tile([128, 256], BF16)      # V_aug tiles 0,1 (bf16, padded)
            v8 = pp.tile([128, n_own * 128], FP8)  # V_aug all tiles (fp8, padded)
            aux_sb = pp.tile([128, aux_w], BF16)
            wkq_sb = aux_sb[:, 0:n_cchunk * 128]
            wv_sb = aux_sb[:, n_cchunk * 128:n_cchunk * 128 + n_cchunk * H]
            mask0 = aux_sb[:, aux_w - 1024:aux_w - 512]
            mask1 = aux_sb[:, aux_w - 512:aux_w]
            aux8_sb = pp.tile([128, aux8_w], FP8)
            wk8_sb = aux8_sb[:, 0:(n_cchunk // 2) * 128]
            mask0_8 = aux8_sb[:, aux8_w - 1024:aux8_w - 512]
            mask1_8 = aux8_sb[:, aux8_w - 512:aux8_w]
            bias_kq = pp.tile([128, 1], F32)
            bias_v = pp.tile([64, 1], F32)
            ident = pp.tile([128, 128], F32)

            make_identity(nc, ident[:, :])

            xt_v = xt[:, :].rearrange("p (s j t) -> p s j t", j=n_cchunk, t=512)
            x8_v = x8[:, :].rearrange(
                "p (s g e t) -> p s g e t", g=n_cchunk // 2, e=2, t=512)
            wk8_v = wk8_sb.rearrange("p (g e h) -> p g e h", e=2, h=H)
            v8_v = v8[:, :].rearrange("p (i c) -> p i c", c=128)

            # wkq gates the first projection: put it on the sync queue AHEAD
            # of the x stream; the first x piece goes on the scalar queue so
            # the two land in parallel. The other small transfers are issued
            # after the load loop (emit_small_dmas).
            nc.sync.dma_start(out=aux_sb[:, 0:n_cchunk * 128],
                              in_=aux_d[:, 0:n_cchunk * 128])

            def emit_small_dmas():
                nc.scalar.dma_start(out=bias_kq[:, 0:1], in_=bkq_d[:, None])
                nc.scalar.dma_start(out=bias_v[:, 0:1], in_=bv_d[:, None])
                nc.scalar.dma_start(out=aux8_sb[:, :], in_=aux8_d)
                nc.scalar.dma_start(out=aux_sb[:, n_cchunk * 128:aux_w],
                                    in_=aux_d[:, n_cchunk * 128:aux_w])

            # V_aug padding: zero cols, ones at col 64 of each slot
            nc.gpsimd.memset(v8[:, :], 0.0)
            nc.gpsimd.memset(v_sb[:, :], 0.0)
            nc.vector.tensor_scalar(
                v8_v[:, :, 64], ident[:, 0:n_own], 0.0, 1.0,
                op0=mybir.AluOpType.mult, op1=mybir.AluOpType.add)
            nc.vector.tensor_scalar(
                v_sb[:, :].rearrange("p (i c) -> p i c", c=128)[:, :, 64],
                ident[:, 0:2], 0.0, 1.0,
                op0=mybir.AluOpType.mult, op1=mybir.AluOpType.add)

            def load_own(sl):
                if sl == 0:
                    # split so the first chunks land (and the first
                    # projection starts) as early as possible
                    nc.scalar.dma_start(out=xt[:, 0:1024],
                                        in_=xt_d[:, 0:1024])
                    nc.sync.dma_start(out=xt[:, 1024:2048],
                                      in_=xt_d[:, 1024:2048])
                    nc.sync.dma_start(out=xt[:, 2048:4096],
                                      in_=xt_d[:, 2048:4096])
                    return
                nc.sync.dma_start(
                    out=xt[:, 4096 * sl:4096 * (sl + 1)],
                    in_=xt_d[:, 4096 * sl:4096 * (sl + 1)])

            def load_partner(sl):
                nc.sync.dma_start(
                    out=x8[:, 4096 * sl:4096 * (sl + 1)],
                    in_=x8_d[:, 4096 * sl:4096 * (sl + 1)])

            def project_kq_own(sl):
                # fused [wk|wq] bf16 projection of own slice sl
                ps = psb.tile([128, 512], F32, tag="psB")
                for j in range(n_cchunk):
                    nc.tensor.matmul(
                        ps[:, :], wkq_sb[:, 128 * j:128 * (j + 1)],
                        xt_v[:, sl, j], start=(j == 0),
                        stop=(j == n_cchunk - 1))
                nc.vector.tensor_scalar_add(
                    kqT[:, 512 * sl:512 * (sl + 1)], ps[:, :], bias_kq[:, 0:1])
                nc.gpsimd.dma_start(
                    sd_hi[64:128, 512 * sl:512 * (sl + 1)],
                    kqT[0:64, 512 * sl:512 * (sl + 1)])
                nc.gpsimd.dma_start(
                    gt_lo[:, 512 * sl:512 * (sl + 1)],
                    kqT[64:128, 512 * sl:512 * (sl + 1)])

            def project_k_partner(sl):
                # fp8 DoubleRow projection: K only, chunk pairs
                ps = psb.tile([64, 512], F32, tag="psB")
                for g in range(n_cchunk // 2):
                    nc.tensor.matmul(
                        ps[:, :], wk8_v[:, g], x8_v[:, sl, g],
                        start=(g == 0), stop=(g == n_cchunk // 2 - 1),
                        perf_mode=mybir.MatmulPerfMode.DoubleRow)
                c0 = t_own + 512 * sl
                nc.vector.tensor_scalar_add(
                    kqT[0:64, c0:c0 + 512], ps[:, :], bias_kq[0:64, 0:1])
                nc.gpsimd.dma_start(
                    sd_hi[64:128, c0:c0 + 512], kqT[0:64, c0:c0 + 512])

            def project_v(sl):
                ps = psb.tile([64, 512], F32, tag="psB")
                for j in range(n_cchunk):
                    nc.tensor.matmul(
                        ps[:, :], wv_sb[:, H * j:H * (j + 1)],
                        xt_v[:, sl, j], start=(j == 0),
                        stop=(j == n_cchunk - 1))
                nc.vector.tensor_scalar_add(
                    vT[:, 512 * sl:512 * (sl + 1)], ps[:, :], bias_v[:, 0:1])

            def v_transpose(i):
                ps = psb.tile([128, 64], F32, tag="psB")
                nc.tensor.transpose(
                    ps[:, :], vT[:, 128 * i:128 * (i + 1)], ident[0:64, 0:64])
                nc.vector.tensor_copy(v8[:, 128 * i:128 * i + 64], ps[:, :])
                if i < 2:
                    nc.vector.tensor_copy(v_sb[:, 128 * i:128 * i + 64],
                                          ps[:, :])

            kq_lo = kqT[0:64, :].rearrange("p (h t) -> p h t", h=2)
            sd_v = sd_hi[64:128, :].rearrange("p (h t) -> p h t", h=2)

            # ---- attention units, software-pipelined with lag-1 PV ----
            # Emitting scores(unit j+1) BEFORE pv(unit j) gives the ACT exp
            # of unit j a full score-pair of tensor time to finish, so the
            # PV matmul never stalls and the PE keeps its pstate/duty.
            po_of = {}

            def emit_scores(tb, ip):
                i0, i1 = 2 * ip, 2 * ip + 1
                ps = pss.tile([128, 1024], F32, tag="psS")
                nc.tensor.matmul(
                    ps[:, 0:512],
                    gt_lo[:, 128 * i0:128 * (i0 + 1)],
                    kq_lo[:, :, 256 * tb:256 * (tb + 1)],
                    start=True, stop=True, tile_position=(0, 0))
                nc.tensor.matmul(
                    ps[:, 512:1024],
                    kqT[64:128, 128 * i1:128 * (i1 + 1)],
                    sd_v[:, :, 256 * tb:256 * (tb + 1)],
                    start=True, stop=True, tile_position=(64, 0))
                return ps

            def emit_exp(tb, ip, ps):
                if tb == 0:
                    pt = ptdp.tile([128, 1024], BF16, tag="ptd")
                else:
                    pt = pt8p.tile([128, 1024], FP8, tag="pt8")
                if ip == tb:
                    # diagonal: cols 512:640 and 768:896 (partner-tile
                    # subtiles above the diagonal for BOTH parities) never
                    # survive the mask -- skip their exp, just zero them
                    nc.scalar.activation(
                        pt[:, 0:512], ps[:, 0:512],
                        mybir.ActivationFunctionType.Exp, scale=SCALE)
                    nc.scalar.activation(
                        pt[:, 640:768], ps[:, 640:768],
                        mybir.ActivationFunctionType.Exp, scale=SCALE)
                    nc.scalar.activation(
                        pt[:, 896:1024], ps[:, 896:1024],
                        mybir.ActivationFunctionType.Exp, scale=SCALE)
                    nc.gpsimd.memset(pt[:, 512:640], 0.0)
                    nc.gpsimd.memset(pt[:, 768:896], 0.0)
                    m0 = mask0 if tb == 0 else mask0_8
                    m1 = mask1 if tb == 0 else mask1_8
                    nc.vector.tensor_mul(pt[:, 0:512], pt[:, 0:512], m0)
                    nc.vector.tensor_mul(pt[:, 640:768], pt[:, 640:768],
                                         m1[:, 128:256])
                    nc.vector.tensor_mul(pt[:, 896:1024], pt[:, 896:1024],
                                         m1[:, 384:512])
                else:
                    nc.scalar.activation(
                        pt[:, :], ps[:, :],
                        mybir.ActivationFunctionType.Exp, scale=SCALE)
                return pt

            def emit_pv(tb, ip, pt):
                if ip == 0:
                    po_of[tb] = pso.tile([128, 512], F32, tag="psO",
                                         name=f"po{tb}")
                po = po_of[tb]
                if tb == 0:
                    # stationary trimmed to 65 cols: the 63 pad columns
                    # would burn PE power the throttler charges us for
                    nc.tensor.matmul(
                        po[0:65, :], v_sb[:, 0:65], pt[:, 0:512],
                        start=True, stop=False)
                    nc.tensor.matmul(
                        po[0:65, :], v_sb[:, 128:193], pt[:, 512:1024],
                        start=False, stop=True)
                else:
                    i0 = 2 * ip
                    nc.tensor.matmul(
                        po[0:65, :],
                        v8[:, 128 * i0:128 * (i0 + 2)].rearrange(
                            "p (e c) -> p e c", e=2)[:, :, 0:65],
                        pt[:, :].rearrange("p (e t) -> p e t", e=2),
                        start=(ip == 0), stop=(ip == tb),
                        perf_mode=mybir.MatmulPerfMode.DoubleRow)
                if ip == tb:
                    ob = outp.tile([65, 512], F32, tag="ob")
                    nc.vector.tensor_copy(ob[:, :], po[0:65, :])
                    # sync HWDGE, not gpsimd SWDGE: the SWDGE ring drain of
                    # the final block costs ~3us at kernel end
                    nc.sync.dma_start(
                        out=out_d[:, 512 * tb:512 * (tb + 1)], in_=ob[:, :])
                    del po_of[tb]

            pending = []

            def stream_unit(tb, ip):
                ps = emit_scores(tb, ip)
                pt = emit_exp(tb, ip, ps)
                pending.append((tb, ip, pt))
                if len(pending) > 2:
                    emit_pv(*pending.pop(0))

            # ---- staged pipeline ----
            for k in range(n_vq):
                load_own(k)
                load_partner(k)
            emit_small_dmas()
            for k in range(n_vq):
                project_kq_own(k)
                project_k_partner(k)
                project_v(k)
                for i in range(4 * k, min(4 * (k + 1), n_own)):
                    v_transpose(i)
                blocks = ((2 * k + 1, 2 * k) if k == n_vq - 1
                          else (2 * k, 2 * k + 1))
                for tb in blocks:
                    for ip in range(tb + 1):
                        stream_unit(tb, ip)
            while pending:
                emit_pv(*pending.pop(0))

    nc.compile()
    return nc


def make_core_inputs(x, Wk, bk, Wq, bq, Wv, bv, t_full=T):
    """Shard FULL inputs into the 8 per-core input dicts (layout prep only)."""
    n_tiles = t_full // 128
    n_cchunk = C // 128
    t_own = t_full // 2
    n_vq = t_own // 512
    Wk = np.asarray(Wk, np.float32)
    Wq = np.asarray(Wq, np.float32)
    Wv = np.asarray(Wv, np.float32)
    wkq = np.empty((128, n_cchunk * 128), np.float32)
    wvf = np.empty((128, n_cchunk * H), np.float32)
    for j in range(n_cchunk):
        wkq[:, 128 * j:128 * j + 64] = Wk[128 * j:128 * (j + 1), :]
        wkq[:, 128 * j + 64:128 * (j + 1)] = Wq[128 * j:128 * (j + 1), :]
        wvf[:, H * j:H * (j + 1)] = Wv[128 * j:128 * (j + 1), :]
    # wk8: chunk pairs (2g, 2g+1) side by side, fp8
    wk8 = np.empty((128, (n_cchunk // 2) * 128), np.float32)
    for g in range(n_cchunk // 2):
        wk8[:, 128 * g:128 * g + 64] = Wk[128 * (2 * g):128 * (2 * g + 1), :]
        wk8[:, 128 * g + 64:128 * (g + 1)] = \
            Wk[128 * (2 * g + 1):128 * (2 * g + 2), :]
    bkq = np.concatenate([np.asarray(bk, np.float32),
                          np.asarray(bq, np.float32)])

    def slice_major(mat):
        # mat [t_own, C] -> [128, n_vq*n_cchunk*512] with
        # out[p, 4096*sl + 512*j + t] = mat[512*sl + t, 128*j + p]
        a = mat.reshape(n_vq, 512, n_cchunk, 128)
        return np.ascontiguousarray(
            a.transpose(3, 0, 2, 1).reshape(128, n_vq * n_cchunk * 512))

    ins = []
    for core in range(N_CORES):
        b, p = core // 2, core % 2
        own = np.concatenate(
            [x[b, 128 * j:128 * (j + 1), :] for j in range(p, n_tiles, 2)],
            axis=0)
        part = np.concatenate(
            [x[b, 128 * j:128 * (j + 1), :] for j in range(1 - p, n_tiles, 2)],
            axis=0)
        # mask[m][r, c]: own s-tile (local parity m, abs tile 4tb+2m+p) vs
        # query sub-tile c//128 (abs tile 4tb + A[c//128]); valid iff s <= t
        A = [p, 2 + p, 1 - p, 3 - p]
        masks = np.zeros((2, 128, 512), np.float32)
        rr = np.arange(128)[:, None]
        for m in (0, 1):
            for sub in range(4):
                cz = np.arange(128)[None, :]
                s_abs = 128 * (2 * m + p) + rr
                t_abs = 128 * A[sub] + cz
                masks[m, :, 128 * sub:128 * (sub + 1)] = (s_abs <= t_abs)
        aux = np.concatenate([wkq, wvf, masks[0], masks[1]], axis=1)
        aux8 = np.concatenate([wk8, masks[0], masks[1]], axis=1)
        ins.append({
            "xt_own": slice_major(own).astype(ml_dtypes.bfloat16),
            "x8_part": slice_major(part).astype(NP_FP8),
            "aux": aux.astype(ml_dtypes.bfloat16),
            "aux8": aux8.astype(NP_FP8),
            "bkq": bkq, "bv": np.asarray(bv, np.float32),
        })
    return ins


def _col_perm(p, t_full):
    """stored column -> absolute t index for a core with parity p."""
    A = [p, 2 + p, 1 - p, 3 - p]
    perm = np.empty(t_full, np.int64)
    for tb in range(t_full // 512):
        for sub in range(4):
            a = 128 * (4 * tb + A[sub])
            s = 512 * tb + 128 * sub
            perm[s:s + 128] = np.arange(a, a + 128)
    return perm


def combine_outputs(parts, t_full=T):
    """parts: list of 8 arrays [H+1, t_full] -> full output [B, t_full, H]."""
    out = np.empty((B, t_full, H), np.float32)
    for b in range(B):
        acc = np.zeros((H + 1, t_full), np.float32)
        for p in (0, 1):
            perm = _col_perm(p, t_full)
            acc[:, perm] += parts[2 * b + p]
        out[b] = (acc[:H, :] / acc[H:H + 1, :]).T
    return out


_NC_CACHE = {}


def kernel(x, Wk, bk, Wq, bq, Wv, bv):
    x = np.asarray(x, np.float32)
    t_full = x.shape[1]
    if t_full not in _NC_CACHE:
        _NC_CACHE[t_full] = build_kernel(t_full)
    nc = _NC_CACHE[t_full]
    ins = make_core_inputs(x, Wk, bk, Wq, bq, Wv, bv, t_full)
    res = run_bass_kernel_spmd(nc, ins, list(range(N_CORES)))
    parts = [res.results[i]["out_part"] for i in range(N_CORES)]
    return combine_outputs(parts, t_full)


if __name__ == "__main__":
    rng = np.random.default_rng(0)
    x = rng.standard_normal((B, T, C), dtype=np.float32)
    Wk = rng.standard_normal((C, H), dtype=np.float32) * SCALE
    Wq = rng.standard_normal((C, H), dtype=np.float32) * SCALE
    Wv = rng.standard_normal((C, H), dtype=np.float32) * SCALE
    bk = rng.standard_normal(H).astype(np.float32) * 0.02
    bq = rng.standard_normal(H).astype(np.float32) * 0.02
    bv = rng.standard_normal(H).astype(np.float32) * 0.02
    out = kernel(x=x, Wk=Wk, bk=bk, Wq=Wq, bq=bq, Wv=Wv, bv=bv)
    print(out.shape, out.dtype)
